# revision 9
# baseline (speedup 1.0000x reference)
"""Expert-parallel MoE SwiGLU FFN kernel for 8 Trainium2 NeuronCores.

Problem: T=4096 tokens, DIM=1024, E=8 experts, INTER=1408, top-2 routing.
Reference computes all experts densely then gathers; we instead route on the
host (sort token-slots by expert), assign one expert per core, and each core
runs a SwiGLU FFN over only its routed tokens (padded to a common capacity so
all 8 cores execute the same SPMD program).

Tokens whose two routed experts coincide are computed once and scattered to
both output slots, which shrinks the per-expert capacity C (~6% of slots are
duplicates for iid top-2 routing).

Device layout (per core, everything "transposed" with tokens on the free dim):
  xt  [8,128,C]  bf16   x_gathered.T tiled over DIM      (k-tile, partition, token)
  w1t [8,128,1408] bf16 w1[e].T tiled over DIM
  w3t [8,128,1408] bf16
  w2t [11,128,1024] bf16 w2[e].T tiled over INTER
  yt  [8,128,C]  bf16   y.T tiled over DIM (output; host upcasts to f32)

Compute per core:
  h1.T = w1 @ x.T   (accumulate over 8 DIM k-tiles)     -> PSUM [128, n]
  h3.T = w3 @ x.T
  g.T  = silu(h1.T) * h3.T                              -> SBUF bf16
  y.T  = w2 @ g.T   (accumulate over 11 INTER m-tiles)  -> PSUM -> SBUF bf16 -> HBM
"""

import numpy as np
import ml_dtypes

T, DIM, E, INTER, TOPK = 4096, 1024, 8, 1408, 2
NCORES = 8
P = 128
KT = DIM // P    # 8 k-tiles over DIM
MT = INTER // P  # 11 m-tiles over INTER

TRACE = False  # test.py sets this to capture an NTFF profile
LAST_RESULTS = None  # BassKernelResults of the last run (for test.py)

_NC_CACHE = {}


def _chunks_for(C):
    # Split C into equal-ish chunks of at most 512 (PSUM bank = 512 fp32),
    # multiples of 16, avoiding a tiny LDWEIGHTS-bound tail chunk.
    nch = -(-C // 512)
    base = C // nch
    out = []
    rem = C
    for i in range(nch, 0, -1):
        n = min(512, -(-rem // i))
        n = -(-n // 16) * 16 if i > 1 else rem  # keep multiples of 16
        n = min(n, 512, rem)
        out.append(n)
        rem -= n
    assert sum(out) == C and all(0 < n <= 512 for n in out), out
    return out


def _build_nc(C):
    import concourse.mybir as mybir
    import concourse.tile as tile
    from concourse import bacc

    dt = mybir.dt
    AF = mybir.ActivationFunctionType
    chunks = _chunks_for(C)

    nc = bacc.Bacc(
        "TRN2", target_bir_lowering=False, debug=False, enable_asserts=False
    )
    # x is stored chunk-major: one contiguous [P, KT, n] block per chunk so a
    # single full-rate DMA delivers each chunk. w1/w3 are m-column-major
    # ([MT, P, KT, 128]) so weight DMAs land in phase-A consumption order.
    xts = [
        nc.dram_tensor(f"xt{j}", [P, KT, n], dt.bfloat16, kind="ExternalInput")
        for j, n in enumerate(chunks)
    ]
    w1t = nc.dram_tensor("w1t", [MT, P, KT * P], dt.bfloat16, kind="ExternalInput")
    w3t = nc.dram_tensor("w3t", [MT, P, KT * P], dt.bfloat16, kind="ExternalInput")
    w2t = nc.dram_tensor("w2t", [MT, P, DIM], dt.bfloat16, kind="ExternalInput")
    yt = nc.dram_tensor("yt", [KT, P, C], dt.bfloat16, kind="ExternalOutput")

    with tile.TileContext(nc) as tc:
        with (
            tc.tile_pool(name="persist", bufs=1) as wpool,
            tc.tile_pool(name="gbuf", bufs=3) as gpool,
            tc.tile_pool(name="ybuf", bufs=4) as ypool,
            tc.tile_pool(name="silbuf", bufs=3) as spool,
            tc.tile_pool(name="psA", bufs=3, space="PSUM") as psA,
            tc.tile_pool(name="psB", bufs=2, space="PSUM") as psB,
        ):
            # SBUF layouts mirror the DRAM layouts so every DMA is contiguous
            # on both sides: per-chunk x tiles, m-major w1/w3.
            xss = [wpool.tile([P, KT, n], dt.bfloat16, name=f"xs{j}")
                   for j, n in enumerate(chunks)]
            w1s = wpool.tile([P, MT, KT * P], dt.bfloat16)
            w3s = wpool.tile([P, MT, KT * P], dt.bfloat16)
            w2s = wpool.tile([P, MT, DIM], dt.bfloat16)
            H = KT * P // 2
            # Each dma_start costs ~650ns of serialized issue time (DIRECT2D)
            # on its queue's engine. Only the sync (SP) and scalar
            # (Activation) queues are fast hardware DGE; gpsimd is a software
            # DGE with ~3us trigger-to-completion latency, so it gets nothing.
            # The m=0 critical-path transfers (w1 m0, all of x chunk 0, w3 m0)
            # are interleaved across sync+scalar so triggers issue in parallel
            # and the first matmul fires ~2us sooner. Everything after m0 only
            # consumes weights at ~150 GB/s, well under DMA rate, so bulk
            # transfers suffice.
            nc.sync.dma_start(w1s[:, 0, :H], w1t[0][:, :H])
            nc.scalar.dma_start(w3s[:, 0, :H], w3t[0][:, :H])
            nc.sync.dma_start(xss[0][:, :2, :], xts[0][:, :2, :])
            nc.scalar.dma_start(xss[0][:, 2:4, :], xts[0][:, 2:4, :])
            nc.sync.dma_start(w1s[:, 0, H:], w1t[0][:, H:])
            nc.scalar.dma_start(w3s[:, 0, H:], w3t[0][:, H:])
            nc.sync.dma_start(xss[0][:, 4:6, :], xts[0][:, 4:6, :])
            nc.scalar.dma_start(xss[0][:, 6:, :], xts[0][:, 6:, :])
            for m in range(1, 4):
                nc.sync.dma_start(w1s[:, m, :], w1t[m])
                nc.scalar.dma_start(w3s[:, m, :], w3t[m])
            for j in range(1, len(chunks)):
                nc.scalar.dma_start(xss[j][:], xts[j][:])
            nc.sync.dma_start(w1s[:, 4:8, :], w1t[4:8].rearrange("m p q -> p m q"))
            nc.scalar.dma_start(w3s[:, 4:8, :], w3t[4:8].rearrange("m p q -> p m q"))
            nc.sync.dma_start(w1s[:, 8:, :], w1t[8:].rearrange("m p q -> p m q"))
            nc.scalar.dma_start(w3s[:, 8:, :], w3t[8:].rearrange("m p q -> p m q"))
            nc.sync.dma_start(w2s[:, :6, :], w2t[:6].rearrange("m p q -> p m q"))
            nc.sync.dma_start(w2s[:, 6:, :], w2t[6:].rearrange("m p q -> p m q"))

            c0 = 0
            nchunks = len(chunks)
            for j, n in enumerate(chunks):
                xsj = xss[j]
                gs = gpool.tile([P, MT, n], dt.bfloat16, name="gs")
                for m in range(MT):
                    p1 = psA.tile([P, n], dt.float32, name="p1")
                    p3 = psA.tile([P, n], dt.float32, name="p3")
                    for k in range(KT):
                        nc.tensor.matmul(
                            p1[:],
                            w1s[:, m, k * P:(k + 1) * P],
                            xsj[:, k, :],
                            start=(k == 0),
                            stop=(k == KT - 1),
                        )
                    for k in range(KT):
                        nc.tensor.matmul(
                            p3[:],
                            w3s[:, m, k * P:(k + 1) * P],
                            xsj[:, k, :],
                            start=(k == 0),
                            stop=(k == KT - 1),
                        )
                    sil = spool.tile([P, n], dt.bfloat16, name="sil")
                    nc.scalar.activation(sil[:], p1[:], AF.Silu)
                    nc.vector.tensor_mul(gs[:, m, :], sil[:], p3[:])
                for i in range(KT):
                    # The very last output tile is split column-wise so the
                    # post-final-matmul chain (CAST + DMA trigger + ring
                    # transfer) only covers a small slice; the big slice
                    # drains while the small slice's matmuls run.
                    last = j == nchunks - 1 and i == KT - 1
                    TAIL = 232  # ~= LDWEIGHTS latency in matmul columns
                    splits = [(0, n - TAIL), (n - TAIL, n)] if (last and n - TAIL >= TAIL) else [(0, n)]
                    for si, (a, b) in enumerate(splits):
                        py = psB.tile([P, b - a], dt.float32, name="py")
                        for m in range(MT):
                            nc.tensor.matmul(
                                py[:],
                                w2s[:, m, i * P:(i + 1) * P],
                                gs[:, m, a:b],
                                start=(m == 0),
                                stop=(m == MT - 1),
                            )
                        ys = ypool.tile([P, b - a], dt.bfloat16, name="ys")
                        nc.vector.tensor_copy(ys[:], py[:])
                        q = nc.sync if (i + si) % 2 == 1 else nc.scalar
                        q.dma_start(yt[i, :, c0 + a:c0 + b], ys[:])
                c0 += n

    nc.compile()
    return nc


def _get_nc(C):
    if C not in _NC_CACHE:
        _NC_CACHE[C] = _build_nc(C)
    return _NC_CACHE[C]


def _ensure_ntff_hook_importable():
    # bass_utils imports antenv.axon_hooks when tracing is requested (e.g. via
    # a BASS_TRACE env var); in containers whose antenv stub lacks that
    # submodule the import would crash. Register a null hook so tracing just
    # degrades to "no trace" instead.
    import sys
    import types

    try:
        import antenv.axon_hooks  # noqa: F401
    except ImportError:
        mod = types.ModuleType("antenv.axon_hooks")
        mod.get_axon_ntff_profile_hook = lambda: None
        mod.set_axon_ntff_profile_hook = lambda hook: None
        sys.modules["antenv.axon_hooks"] = mod


def kernel(x, expert_indices, w1, w2, w3):
    global LAST_RESULTS
    import os
    import sys

    # The bass kernel executes on the NeuronCores via the axon PJRT backend;
    # a JAX_PLATFORMS=cpu pin (commonly used for running jax reference code)
    # would hide those devices. Clear it if jax hasn't initialized yet.
    if os.environ.get("JAX_PLATFORMS") == "cpu" and "jax" not in sys.modules:
        del os.environ["JAX_PLATFORMS"]

    from concourse import bass_utils

    _ensure_ntff_hook_importable()
    x = np.asarray(x, dtype=np.float32)
    idx = np.asarray(expert_indices)
    w1 = np.asarray(w1, dtype=np.float32)
    w2 = np.asarray(w2, dtype=np.float32)
    w3 = np.asarray(w3, dtype=np.float32)

    bf16 = ml_dtypes.bfloat16

    # --- host routing: stable-sort the (token, k) slots by expert id,
    # dropping slots whose (token, expert) pair duplicates slot k=0 ---
    flat = idx.reshape(-1).astype(np.int64)  # slot s = t*TOPK + k -> expert
    keep = np.ones(T * TOPK, dtype=bool)
    dup = idx[:, 1] == idx[:, 0]
    keep[1::2] = ~dup
    kept_slots = np.nonzero(keep)[0]
    kept_flat = flat[keep]
    order = np.argsort(kept_flat, kind="stable")  # kept slots grouped by expert
    sorted_slots = kept_slots[order]
    counts = np.bincount(kept_flat, minlength=E)
    starts = np.zeros(E + 1, dtype=np.int64)
    np.cumsum(counts, out=starts[1:])
    cmax = int(counts.max())
    C = max(256, -(-cmax // 8) * 8)  # pad capacity to a multiple of 8

    nc = _get_nc(C)

    chunks = _chunks_for(C)
    bounds = np.cumsum([0] + chunks)
    xb = x.astype(bf16)
    in_maps = []
    for e in range(E):
        slots = sorted_slots[starts[e]:starts[e + 1]]
        tokens = slots // TOPK
        xg = np.zeros((C, DIM), dtype=bf16)
        xg[: len(tokens)] = xb[tokens]
        # [C, DIM] -> [P, KT, C] (partition-major), then per-chunk blocks
        xpkc = xg.T.reshape(KT, P, C).transpose(1, 0, 2)
        im = {
            f"xt{j}": np.ascontiguousarray(xpkc[:, :, bounds[j]:bounds[j + 1]])
            for j in range(len(chunks))
        }
        # w1t[m, p, k*128+j] = w1[e][m*128+j, k*128+p]
        im["w1t"] = np.ascontiguousarray(
            w1[e].astype(bf16).reshape(MT, P, KT, P).transpose(0, 3, 2, 1)
        ).reshape(MT, P, KT * P)
        im["w3t"] = np.ascontiguousarray(
            w3[e].astype(bf16).reshape(MT, P, KT, P).transpose(0, 3, 2, 1)
        ).reshape(MT, P, KT * P)
        im["w2t"] = np.ascontiguousarray(w2[e].T.astype(bf16)).reshape(MT, P, DIM)
        in_maps.append(im)

    res = bass_utils.run_bass_kernel_spmd(
        nc, in_maps, core_ids=list(range(NCORES)), trace=TRACE
    )
    LAST_RESULTS = res

    out = np.empty((T * TOPK, DIM), dtype=np.float32)
    for e in range(E):
        slots = sorted_slots[starts[e]:starts[e + 1]]
        yt = res.results[e]["yt"]  # [KT, P, C] bf16
        y = yt.reshape(DIM, C).astype(np.float32)  # y.T
        out[slots] = y[:, : len(slots)].T
    out = out.reshape(T, TOPK, DIM)
    out[dup, 1] = out[dup, 0]  # slots dropped by dedupe share the k=0 result
    return out


# revision 11
# speedup vs baseline: 1.0117x; 1.0117x over previous
"""Expert-parallel MoE SwiGLU FFN kernel for 8 Trainium2 NeuronCores.

Problem: T=4096 tokens, DIM=1024, E=8 experts, INTER=1408, top-2 routing.
Reference computes all experts densely then gathers; we instead route on the
host (sort token-slots by expert), assign one expert per core, and each core
runs a SwiGLU FFN over only its routed tokens (padded to a common capacity so
all 8 cores execute the same SPMD program).

Tokens whose two routed experts coincide are computed once and scattered to
both output slots, which shrinks the per-expert capacity C (~6% of slots are
duplicates for iid top-2 routing).

Device layout (per core, everything "transposed" with tokens on the free dim):
  xt  [8,128,C]  bf16   x_gathered.T tiled over DIM      (k-tile, partition, token)
  w1t [8,128,1408] bf16 w1[e].T tiled over DIM
  w3t [8,128,1408] bf16
  w2t [11,128,1024] bf16 w2[e].T tiled over INTER
  yt  [8,128,C]  bf16   y.T tiled over DIM (output; host upcasts to f32)

Compute per core:
  h1.T = w1 @ x.T   (accumulate over 8 DIM k-tiles)     -> PSUM [128, n]
  h3.T = w3 @ x.T
  g.T  = silu(h1.T) * h3.T                              -> SBUF bf16
  y.T  = w2 @ g.T   (accumulate over 11 INTER m-tiles)  -> PSUM -> SBUF bf16 -> HBM
"""

import numpy as np
import ml_dtypes

T, DIM, E, INTER, TOPK = 4096, 1024, 8, 1408, 2
NCORES = 8
P = 128
KT = DIM // P    # 8 k-tiles over DIM
MT = INTER // P  # 11 m-tiles over INTER

TRACE = False  # test.py sets this to capture an NTFF profile
LAST_RESULTS = None  # BassKernelResults of the last run (for test.py)

_NC_CACHE = {}


def _chunks_for(C):
    # Split C into equal-ish chunks of at most 512 (PSUM bank = 512 fp32),
    # multiples of 16, avoiding a tiny LDWEIGHTS-bound tail chunk.
    nch = -(-C // 512)
    base = C // nch
    out = []
    rem = C
    for i in range(nch, 0, -1):
        n = min(512, -(-rem // i))
        n = -(-n // 16) * 16 if i > 1 else rem  # keep multiples of 16
        n = min(n, 512, rem)
        out.append(n)
        rem -= n
    assert sum(out) == C and all(0 < n <= 512 for n in out), out
    return out


def _build_nc(C):
    import concourse.mybir as mybir
    import concourse.tile as tile
    from concourse import bacc

    dt = mybir.dt
    AF = mybir.ActivationFunctionType
    chunks = _chunks_for(C)

    nc = bacc.Bacc(
        "TRN2", target_bir_lowering=False, debug=False, enable_asserts=False
    )
    # x is stored chunk-major: one contiguous [P, KT, n] block per chunk so a
    # single full-rate DMA delivers each chunk. w1/w3 are m-column-major
    # ([MT, P, KT, 128]) so weight DMAs land in phase-A consumption order.
    xts = [
        nc.dram_tensor(f"xt{j}", [P, KT, n], dt.bfloat16, kind="ExternalInput")
        for j, n in enumerate(chunks)
    ]
    w1t = nc.dram_tensor("w1t", [MT, P, KT * P], dt.bfloat16, kind="ExternalInput")
    w3t = nc.dram_tensor("w3t", [MT, P, KT * P], dt.bfloat16, kind="ExternalInput")
    w2t = nc.dram_tensor("w2t", [MT, P, DIM], dt.bfloat16, kind="ExternalInput")
    yt = nc.dram_tensor("yt", [KT, P, C], dt.bfloat16, kind="ExternalOutput")

    with tile.TileContext(nc) as tc:
        with (
            tc.tile_pool(name="persist", bufs=1) as wpool,
            tc.tile_pool(name="gbuf", bufs=3) as gpool,
            tc.tile_pool(name="ybuf", bufs=4) as ypool,
            tc.tile_pool(name="silbuf", bufs=3) as spool,
            tc.tile_pool(name="psA", bufs=3, space="PSUM") as psA,
            tc.tile_pool(name="psB", bufs=2, space="PSUM") as psB,
        ):
            # SBUF layouts mirror the DRAM layouts so every DMA is contiguous
            # on both sides: per-chunk x tiles, m-major w1/w3.
            xss = [wpool.tile([P, KT, n], dt.bfloat16, name=f"xs{j}")
                   for j, n in enumerate(chunks)]
            w1s = wpool.tile([P, MT, KT * P], dt.bfloat16)
            w3s = wpool.tile([P, MT, KT * P], dt.bfloat16)
            w2s = wpool.tile([P, MT, DIM], dt.bfloat16)
            H = KT * P // 2
            # The head is ring-bandwidth-bound (~200 GB/s while ramping), so
            # what matters is enqueuing the m=0 critical bytes (w1 m0, x
            # chunk 0, w3 m0) with nothing else ahead of them. All inputs go
            # on the sync queue in consumption order; spreading them over two
            # queues just front-loads weight bytes ahead of x and delays the
            # first matmuls.
            nc.sync.dma_start(w1s[:, 0, :H], w1t[0][:, :H])
            nc.sync.dma_start(xss[0][:, :2, :], xts[0][:, :2, :])
            nc.sync.dma_start(w1s[:, 0, H:], w1t[0][:, H:])
            nc.sync.dma_start(xss[0][:, 2:4, :], xts[0][:, 2:4, :])
            nc.sync.dma_start(w3s[:, 0, :], w3t[0])
            nc.sync.dma_start(xss[0][:, 4:6, :], xts[0][:, 4:6, :])
            nc.sync.dma_start(xss[0][:, 6:, :], xts[0][:, 6:, :])
            for m in range(1, 4):
                nc.sync.dma_start(w1s[:, m, :], w1t[m])
                nc.sync.dma_start(w3s[:, m, :], w3t[m])
            for j in range(1, len(chunks)):
                nc.sync.dma_start(xss[j][:], xts[j][:])
            nc.sync.dma_start(w1s[:, 4:8, :], w1t[4:8].rearrange("m p q -> p m q"))
            nc.sync.dma_start(w3s[:, 4:8, :], w3t[4:8].rearrange("m p q -> p m q"))
            nc.sync.dma_start(w1s[:, 8:, :], w1t[8:].rearrange("m p q -> p m q"))
            nc.sync.dma_start(w3s[:, 8:, :], w3t[8:].rearrange("m p q -> p m q"))
            nc.sync.dma_start(w2s[:, :6, :], w2t[:6].rearrange("m p q -> p m q"))
            nc.sync.dma_start(w2s[:, 6:, :], w2t[6:].rearrange("m p q -> p m q"))

            c0 = 0
            nchunks = len(chunks)
            for j, n in enumerate(chunks):
                xsj = xss[j]
                gs = gpool.tile([P, MT, n], dt.bfloat16, name="gs")
                for m in range(MT):
                    p1 = psA.tile([P, n], dt.float32, name="p1")
                    p3 = psA.tile([P, n], dt.float32, name="p3")
                    for k in range(KT):
                        nc.tensor.matmul(
                            p1[:],
                            w1s[:, m, k * P:(k + 1) * P],
                            xsj[:, k, :],
                            start=(k == 0),
                            stop=(k == KT - 1),
                        )
                    for k in range(KT):
                        nc.tensor.matmul(
                            p3[:],
                            w3s[:, m, k * P:(k + 1) * P],
                            xsj[:, k, :],
                            start=(k == 0),
                            stop=(k == KT - 1),
                        )
                    sil = spool.tile([P, n], dt.bfloat16, name="sil")
                    nc.scalar.activation(sil[:], p1[:], AF.Silu)
                    nc.vector.tensor_mul(gs[:, m, :], sil[:], p3[:])
                for i in range(KT):
                    # The very last output tile is split column-wise so the
                    # post-final-matmul chain (CAST + DMA trigger + ring
                    # transfer) only covers a small slice; the big slice
                    # drains while the small slice's matmuls run.
                    last = j == nchunks - 1 and i == KT - 1
                    TAIL = 232  # ~= LDWEIGHTS latency in matmul columns
                    splits = [(0, n - TAIL), (n - TAIL, n)] if (last and n - TAIL >= TAIL) else [(0, n)]
                    for si, (a, b) in enumerate(splits):
                        py = psB.tile([P, b - a], dt.float32, name="py")
                        for m in range(MT):
                            nc.tensor.matmul(
                                py[:],
                                w2s[:, m, i * P:(i + 1) * P],
                                gs[:, m, a:b],
                                start=(m == 0),
                                stop=(m == MT - 1),
                            )
                        ys = ypool.tile([P, b - a], dt.bfloat16, name="ys")
                        nc.vector.tensor_copy(ys[:], py[:])
                        q = nc.sync if (i + si) % 2 == 1 else nc.scalar
                        q.dma_start(yt[i, :, c0 + a:c0 + b], ys[:])
                c0 += n

    nc.compile()
    return nc


def _get_nc(C):
    if C not in _NC_CACHE:
        _NC_CACHE[C] = _build_nc(C)
    return _NC_CACHE[C]


def _ensure_ntff_hook_importable():
    # bass_utils imports antenv.axon_hooks when tracing is requested (e.g. via
    # a BASS_TRACE env var); in containers whose antenv stub lacks that
    # submodule the import would crash. Register a null hook so tracing just
    # degrades to "no trace" instead.
    import sys
    import types

    try:
        import antenv.axon_hooks  # noqa: F401
    except ImportError:
        mod = types.ModuleType("antenv.axon_hooks")
        mod.get_axon_ntff_profile_hook = lambda: None
        mod.set_axon_ntff_profile_hook = lambda hook: None
        sys.modules["antenv.axon_hooks"] = mod


def kernel(x, expert_indices, w1, w2, w3):
    global LAST_RESULTS
    import os
    import sys

    # The bass kernel executes on the NeuronCores via the axon PJRT backend;
    # a JAX_PLATFORMS=cpu pin (commonly used for running jax reference code)
    # would hide those devices. Clear it if jax hasn't initialized yet.
    if os.environ.get("JAX_PLATFORMS") == "cpu" and "jax" not in sys.modules:
        del os.environ["JAX_PLATFORMS"]

    from concourse import bass_utils

    _ensure_ntff_hook_importable()
    x = np.asarray(x, dtype=np.float32)
    idx = np.asarray(expert_indices)
    w1 = np.asarray(w1, dtype=np.float32)
    w2 = np.asarray(w2, dtype=np.float32)
    w3 = np.asarray(w3, dtype=np.float32)

    bf16 = ml_dtypes.bfloat16

    # --- host routing: stable-sort the (token, k) slots by expert id,
    # dropping slots whose (token, expert) pair duplicates slot k=0 ---
    flat = idx.reshape(-1).astype(np.int64)  # slot s = t*TOPK + k -> expert
    keep = np.ones(T * TOPK, dtype=bool)
    dup = idx[:, 1] == idx[:, 0]
    keep[1::2] = ~dup
    kept_slots = np.nonzero(keep)[0]
    kept_flat = flat[keep]
    order = np.argsort(kept_flat, kind="stable")  # kept slots grouped by expert
    sorted_slots = kept_slots[order]
    counts = np.bincount(kept_flat, minlength=E)
    starts = np.zeros(E + 1, dtype=np.int64)
    np.cumsum(counts, out=starts[1:])
    cmax = int(counts.max())
    C = max(256, -(-cmax // 8) * 8)  # pad capacity to a multiple of 8

    nc = _get_nc(C)

    chunks = _chunks_for(C)
    bounds = np.cumsum([0] + chunks)
    xb = x.astype(bf16)
    in_maps = []
    for e in range(E):
        slots = sorted_slots[starts[e]:starts[e + 1]]
        tokens = slots // TOPK
        xg = np.zeros((C, DIM), dtype=bf16)
        xg[: len(tokens)] = xb[tokens]
        # [C, DIM] -> [P, KT, C] (partition-major), then per-chunk blocks
        xpkc = xg.T.reshape(KT, P, C).transpose(1, 0, 2)
        im = {
            f"xt{j}": np.ascontiguousarray(xpkc[:, :, bounds[j]:bounds[j + 1]])
            for j in range(len(chunks))
        }
        # w1t[m, p, k*128+j] = w1[e][m*128+j, k*128+p]
        im["w1t"] = np.ascontiguousarray(
            w1[e].astype(bf16).reshape(MT, P, KT, P).transpose(0, 3, 2, 1)
        ).reshape(MT, P, KT * P)
        im["w3t"] = np.ascontiguousarray(
            w3[e].astype(bf16).reshape(MT, P, KT, P).transpose(0, 3, 2, 1)
        ).reshape(MT, P, KT * P)
        im["w2t"] = np.ascontiguousarray(w2[e].T.astype(bf16)).reshape(MT, P, DIM)
        in_maps.append(im)

    res = bass_utils.run_bass_kernel_spmd(
        nc, in_maps, core_ids=list(range(NCORES)), trace=TRACE
    )
    LAST_RESULTS = res

    out = np.empty((T * TOPK, DIM), dtype=np.float32)
    for e in range(E):
        slots = sorted_slots[starts[e]:starts[e + 1]]
        yt = res.results[e]["yt"]  # [KT, P, C] bf16
        y = yt.reshape(DIM, C).astype(np.float32)  # y.T
        out[slots] = y[:, : len(slots)].T
    out = out.reshape(T, TOPK, DIM)
    out[dup, 1] = out[dup, 0]  # slots dropped by dedupe share the k=0 result
    return out


# revision 15
# speedup vs baseline: 1.0229x; 1.0111x over previous
"""Expert-parallel MoE SwiGLU FFN kernel for 8 Trainium2 NeuronCores.

Problem: T=4096 tokens, DIM=1024, E=8 experts, INTER=1408, top-2 routing.
Reference computes all experts densely then gathers; we instead route on the
host (sort token-slots by expert), assign one expert per core, and each core
runs a SwiGLU FFN over only its routed tokens (padded to a common capacity so
all 8 cores execute the same SPMD program).

Tokens whose two routed experts coincide are computed once and scattered to
both output slots, which shrinks the per-expert capacity C (~6% of slots are
duplicates for iid top-2 routing).

Device layout (per core, everything "transposed" with tokens on the free dim):
  xt  [8,128,C]  bf16   x_gathered.T tiled over DIM      (k-tile, partition, token)
  w1t [8,128,1408] bf16 w1[e].T tiled over DIM
  w3t [8,128,1408] bf16
  w2t [11,128,1024] bf16 w2[e].T tiled over INTER
  yt  [8,128,C]  bf16   y.T tiled over DIM (output; host upcasts to f32)

Compute per core:
  h1.T = w1 @ x.T   (accumulate over 8 DIM k-tiles)     -> PSUM [128, n]
  h3.T = w3 @ x.T
  g.T  = silu(h1.T) * h3.T                              -> SBUF bf16
  y.T  = w2 @ g.T   (accumulate over 11 INTER m-tiles)  -> PSUM -> SBUF bf16 -> HBM
"""

import numpy as np
import ml_dtypes

T, DIM, E, INTER, TOPK = 4096, 1024, 8, 1408, 2
NCORES = 8
P = 128
KT = DIM // P    # 8 k-tiles over DIM
MT = INTER // P  # 11 m-tiles over INTER

TRACE = False  # test.py sets this to capture an NTFF profile
LAST_RESULTS = None  # BassKernelResults of the last run (for test.py)

_NC_CACHE = {}


def _chunks_for(C):
    # Split C into equal-ish chunks of at most 512 (PSUM bank = 512 fp32),
    # multiples of 16, avoiding a tiny LDWEIGHTS-bound tail chunk.
    nch = -(-C // 512)
    base = C // nch
    out = []
    rem = C
    for i in range(nch, 0, -1):
        n = min(512, -(-rem // i))
        n = -(-n // 16) * 16 if i > 1 else rem  # keep multiples of 16
        n = min(n, 512, rem)
        out.append(n)
        rem -= n
    assert sum(out) == C and all(0 < n <= 512 for n in out), out
    return out


def _build_nc(C):
    import concourse.mybir as mybir
    import concourse.tile as tile
    from concourse import bacc

    dt = mybir.dt
    AF = mybir.ActivationFunctionType
    chunks = _chunks_for(C)

    nc = bacc.Bacc(
        "TRN2", target_bir_lowering=False, debug=False, enable_asserts=False
    )
    # x is stored chunk-major: one contiguous [P, KT, n] block per chunk so a
    # single full-rate DMA delivers each chunk. w1/w3 are m-column-major
    # ([MT, P, KT, 128]) so weight DMAs land in phase-A consumption order.
    xts = [
        nc.dram_tensor(f"xt{j}", [P, KT, n], dt.bfloat16, kind="ExternalInput")
        for j, n in enumerate(chunks)
    ]
    w1t = nc.dram_tensor("w1t", [MT, P, KT * P], dt.bfloat16, kind="ExternalInput")
    w3t = nc.dram_tensor("w3t", [MT, P, KT * P], dt.bfloat16, kind="ExternalInput")
    w2t = nc.dram_tensor("w2t", [MT, P, DIM], dt.bfloat16, kind="ExternalInput")
    yt = nc.dram_tensor("yt", [KT, P, C], dt.bfloat16, kind="ExternalOutput")

    with tile.TileContext(nc) as tc:
        with (
            tc.tile_pool(name="persist", bufs=1) as wpool,
            tc.tile_pool(name="gbuf", bufs=3) as gpool,
            tc.tile_pool(name="ybuf", bufs=4) as ypool,
            tc.tile_pool(name="silbuf", bufs=3) as spool,
            tc.tile_pool(name="psA", bufs=3, space="PSUM") as psA,
            tc.tile_pool(name="psB", bufs=2, space="PSUM") as psB,
        ):
            # SBUF layouts mirror the DRAM layouts so every DMA is contiguous
            # on both sides: per-chunk x tiles, m-major w1/w3.
            xss = [wpool.tile([P, KT, n], dt.bfloat16, name=f"xs{j}")
                   for j, n in enumerate(chunks)]
            w1s = wpool.tile([P, MT, KT * P], dt.bfloat16)
            w3s = wpool.tile([P, MT, KT * P], dt.bfloat16)
            w2s = wpool.tile([P, MT, DIM], dt.bfloat16)
            H = KT * P // 2
            # The head is ring-bandwidth-bound (~200 GB/s while ramping), so
            # what matters is enqueuing the m=0 critical bytes (w1 m0, x
            # chunk 0, w3 m0) with nothing else ahead of them. All inputs go
            # on the sync queue in consumption order; spreading them over two
            # queues just front-loads weight bytes ahead of x and delays the
            # first matmuls.
            nc.sync.dma_start(w1s[:, 0, :H], w1t[0][:, :H])
            nc.sync.dma_start(xss[0][:, :2, :], xts[0][:, :2, :])
            nc.sync.dma_start(w3s[:, 0, :H], w3t[0][:, :H])
            nc.sync.dma_start(xss[0][:, 2:4, :], xts[0][:, 2:4, :])
            nc.sync.dma_start(w1s[:, 0, H:], w1t[0][:, H:])
            nc.sync.dma_start(xss[0][:, 4:6, :], xts[0][:, 4:6, :])
            nc.sync.dma_start(w3s[:, 0, H:], w3t[0][:, H:])
            nc.sync.dma_start(xss[0][:, 6:, :], xts[0][:, 6:, :])
            for m in range(1, 4):
                nc.sync.dma_start(w1s[:, m, :], w1t[m])
                nc.sync.dma_start(w3s[:, m, :], w3t[m])
            for j in range(1, len(chunks)):
                nc.sync.dma_start(xss[j][:], xts[j][:])
            nc.sync.dma_start(w1s[:, 4:8, :], w1t[4:8].rearrange("m p q -> p m q"))
            nc.sync.dma_start(w3s[:, 4:8, :], w3t[4:8].rearrange("m p q -> p m q"))
            nc.sync.dma_start(w1s[:, 8:, :], w1t[8:].rearrange("m p q -> p m q"))
            nc.sync.dma_start(w3s[:, 8:, :], w3t[8:].rearrange("m p q -> p m q"))
            nc.sync.dma_start(w2s[:, :6, :], w2t[:6].rearrange("m p q -> p m q"))
            nc.sync.dma_start(w2s[:, 6:, :], w2t[6:].rearrange("m p q -> p m q"))

            c0 = 0
            nchunks = len(chunks)
            for j, n in enumerate(chunks):
                xsj = xss[j]
                gs = gpool.tile([P, MT, n], dt.bfloat16, name="gs")
                for m in range(MT):
                    p1 = psA.tile([P, n], dt.float32, name="p1")
                    p3 = psA.tile([P, n], dt.float32, name="p3")
                    if j == 0 and m == 0:
                        # First tile pair: emit p1/p3 interleaved in the exact
                        # order their DMA dependencies land during the head
                        # ramp, so the PE computes through the ramp instead of
                        # idling until all of w1m0/w3m0/x0 has arrived.
                        seq = []
                        for kk in range(0, KT, 2):
                            seq += [(p1, w1s, kk), (p1, w1s, kk + 1),
                                    (p3, w3s, kk), (p3, w3s, kk + 1)]
                    else:
                        seq = [(p1, w1s, k) for k in range(KT)] + \
                              [(p3, w3s, k) for k in range(KT)]
                    seen = {}
                    for dst, wsrc, k in seq:
                        first = id(dst) not in seen
                        seen[id(dst)] = seen.get(id(dst), 0) + 1
                        nc.tensor.matmul(
                            dst[:],
                            wsrc[:, m, k * P:(k + 1) * P],
                            xsj[:, k, :],
                            start=first,
                            stop=(seen[id(dst)] == KT),
                        )
                    sil = spool.tile([P, n], dt.bfloat16, name="sil")
                    nc.scalar.activation(sil[:], p1[:], AF.Silu)
                    nc.vector.tensor_mul(gs[:, m, :], sil[:], p3[:])
                for i in range(KT):
                    # The very last output tile is split column-wise so the
                    # post-final-matmul chain (CAST + DMA trigger + ring
                    # transfer) only covers a small slice; the big slice
                    # drains while the small slice's matmuls run.
                    last = j == nchunks - 1 and i == KT - 1
                    TAIL = 232  # ~= LDWEIGHTS latency in matmul columns
                    splits = [(0, n - TAIL), (n - TAIL, n)] if (last and n - TAIL >= TAIL) else [(0, n)]
                    for si, (a, b) in enumerate(splits):
                        py = psB.tile([P, b - a], dt.float32, name="py")
                        for m in range(MT):
                            nc.tensor.matmul(
                                py[:],
                                w2s[:, m, i * P:(i + 1) * P],
                                gs[:, m, a:b],
                                start=(m == 0),
                                stop=(m == MT - 1),
                            )
                        ys = ypool.tile([P, b - a], dt.bfloat16, name="ys")
                        nc.vector.tensor_copy(ys[:], py[:])
                        q = nc.sync if (i + si) % 2 == 1 else nc.scalar
                        q.dma_start(yt[i, :, c0 + a:c0 + b], ys[:])
                c0 += n

    nc.compile()
    return nc


def _get_nc(C):
    if C not in _NC_CACHE:
        _NC_CACHE[C] = _build_nc(C)
    return _NC_CACHE[C]


def _ensure_ntff_hook_importable():
    # bass_utils imports antenv.axon_hooks when tracing is requested (e.g. via
    # a BASS_TRACE env var); in containers whose antenv stub lacks that
    # submodule the import would crash. Register a null hook so tracing just
    # degrades to "no trace" instead.
    import sys
    import types

    try:
        import antenv.axon_hooks  # noqa: F401
    except ImportError:
        mod = types.ModuleType("antenv.axon_hooks")
        mod.get_axon_ntff_profile_hook = lambda: None
        mod.set_axon_ntff_profile_hook = lambda hook: None
        sys.modules["antenv.axon_hooks"] = mod


def kernel(x, expert_indices, w1, w2, w3):
    global LAST_RESULTS
    import os
    import sys

    # The bass kernel executes on the NeuronCores via the axon PJRT backend;
    # a JAX_PLATFORMS=cpu pin (commonly used for running jax reference code)
    # would hide those devices. Clear it if jax hasn't initialized yet.
    if os.environ.get("JAX_PLATFORMS") == "cpu" and "jax" not in sys.modules:
        del os.environ["JAX_PLATFORMS"]

    from concourse import bass_utils

    _ensure_ntff_hook_importable()
    x = np.asarray(x, dtype=np.float32)
    idx = np.asarray(expert_indices)
    w1 = np.asarray(w1, dtype=np.float32)
    w2 = np.asarray(w2, dtype=np.float32)
    w3 = np.asarray(w3, dtype=np.float32)

    bf16 = ml_dtypes.bfloat16

    # --- host routing: stable-sort the (token, k) slots by expert id,
    # dropping slots whose (token, expert) pair duplicates slot k=0 ---
    flat = idx.reshape(-1).astype(np.int64)  # slot s = t*TOPK + k -> expert
    keep = np.ones(T * TOPK, dtype=bool)
    dup = idx[:, 1] == idx[:, 0]
    keep[1::2] = ~dup
    kept_slots = np.nonzero(keep)[0]
    kept_flat = flat[keep]
    order = np.argsort(kept_flat, kind="stable")  # kept slots grouped by expert
    sorted_slots = kept_slots[order]
    counts = np.bincount(kept_flat, minlength=E)
    starts = np.zeros(E + 1, dtype=np.int64)
    np.cumsum(counts, out=starts[1:])
    cmax = int(counts.max())
    C = max(256, -(-cmax // 8) * 8)  # pad capacity to a multiple of 8

    nc = _get_nc(C)

    chunks = _chunks_for(C)
    bounds = np.cumsum([0] + chunks)
    xb = x.astype(bf16)
    in_maps = []
    for e in range(E):
        slots = sorted_slots[starts[e]:starts[e + 1]]
        tokens = slots // TOPK
        xg = np.zeros((C, DIM), dtype=bf16)
        xg[: len(tokens)] = xb[tokens]
        # [C, DIM] -> [P, KT, C] (partition-major), then per-chunk blocks
        xpkc = xg.T.reshape(KT, P, C).transpose(1, 0, 2)
        im = {
            f"xt{j}": np.ascontiguousarray(xpkc[:, :, bounds[j]:bounds[j + 1]])
            for j in range(len(chunks))
        }
        # w1t[m, p, k*128+j] = w1[e][m*128+j, k*128+p]
        im["w1t"] = np.ascontiguousarray(
            w1[e].astype(bf16).reshape(MT, P, KT, P).transpose(0, 3, 2, 1)
        ).reshape(MT, P, KT * P)
        im["w3t"] = np.ascontiguousarray(
            w3[e].astype(bf16).reshape(MT, P, KT, P).transpose(0, 3, 2, 1)
        ).reshape(MT, P, KT * P)
        im["w2t"] = np.ascontiguousarray(w2[e].T.astype(bf16)).reshape(MT, P, DIM)
        in_maps.append(im)

    res = bass_utils.run_bass_kernel_spmd(
        nc, in_maps, core_ids=list(range(NCORES)), trace=TRACE
    )
    LAST_RESULTS = res

    out = np.empty((T * TOPK, DIM), dtype=np.float32)
    for e in range(E):
        slots = sorted_slots[starts[e]:starts[e + 1]]
        yt = res.results[e]["yt"]  # [KT, P, C] bf16
        y = yt.reshape(DIM, C).astype(np.float32)  # y.T
        out[slots] = y[:, : len(slots)].T
    out = out.reshape(T, TOPK, DIM)
    out[dup, 1] = out[dup, 0]  # slots dropped by dedupe share the k=0 result
    return out


# revision 18
# speedup vs baseline: 1.0294x; 1.0063x over previous
"""Expert-parallel MoE SwiGLU FFN kernel for 8 Trainium2 NeuronCores.

Problem: T=4096 tokens, DIM=1024, E=8 experts, INTER=1408, top-2 routing.
Reference computes all experts densely then gathers; we instead route on the
host (sort token-slots by expert), assign one expert per core, and each core
runs a SwiGLU FFN over only its routed tokens (padded to a common capacity so
all 8 cores execute the same SPMD program).

Tokens whose two routed experts coincide are computed once and scattered to
both output slots, which shrinks the per-expert capacity C (~6% of slots are
duplicates for iid top-2 routing).

Device layout (per core, everything "transposed" with tokens on the free dim):
  xt  [8,128,C]  bf16   x_gathered.T tiled over DIM      (k-tile, partition, token)
  w1t [8,128,1408] bf16 w1[e].T tiled over DIM
  w3t [8,128,1408] bf16
  w2t [11,128,1024] bf16 w2[e].T tiled over INTER
  yt  [8,128,C]  bf16   y.T tiled over DIM (output; host upcasts to f32)

Compute per core:
  h1.T = w1 @ x.T   (accumulate over 8 DIM k-tiles)     -> PSUM [128, n]
  h3.T = w3 @ x.T
  g.T  = silu(h1.T) * h3.T                              -> SBUF bf16
  y.T  = w2 @ g.T   (accumulate over 11 INTER m-tiles)  -> PSUM -> SBUF bf16 -> HBM
"""

import numpy as np
import ml_dtypes

T, DIM, E, INTER, TOPK = 4096, 1024, 8, 1408, 2
NCORES = 8
P = 128
KT = DIM // P    # 8 k-tiles over DIM
MT = INTER // P  # 11 m-tiles over INTER

TRACE = False  # test.py sets this to capture an NTFF profile
LAST_RESULTS = None  # BassKernelResults of the last run (for test.py)

_NC_CACHE = {}


def _chunks_for(C):
    # Split C into equal-ish chunks of at most 512 (PSUM bank = 512 fp32),
    # multiples of 16, avoiding a tiny LDWEIGHTS-bound tail chunk.
    nch = -(-C // 512)
    base = C // nch
    out = []
    rem = C
    for i in range(nch, 0, -1):
        n = min(512, -(-rem // i))
        n = -(-n // 16) * 16 if i > 1 else rem  # keep multiples of 16
        n = min(n, 512, rem)
        out.append(n)
        rem -= n
    assert sum(out) == C and all(0 < n <= 512 for n in out), out
    return out


def _build_nc(C):
    import concourse.mybir as mybir
    import concourse.tile as tile
    from concourse import bacc

    dt = mybir.dt
    AF = mybir.ActivationFunctionType
    chunks = _chunks_for(C)

    nc = bacc.Bacc(
        "TRN2", target_bir_lowering=False, debug=False, enable_asserts=False
    )
    # x is stored chunk-major: one contiguous [P, KT, n] block per chunk so a
    # single full-rate DMA delivers each chunk. w1/w3 are m-column-major
    # ([MT, P, KT, 128]) so weight DMAs land in phase-A consumption order.
    xts = [
        nc.dram_tensor(f"xt{j}", [P, KT, n], dt.bfloat16, kind="ExternalInput")
        for j, n in enumerate(chunks)
    ]
    w1t = nc.dram_tensor("w1t", [MT, P, KT * P], dt.bfloat16, kind="ExternalInput")
    w3t = nc.dram_tensor("w3t", [MT, P, KT * P], dt.bfloat16, kind="ExternalInput")
    w2t = nc.dram_tensor("w2t", [MT, P, DIM], dt.bfloat16, kind="ExternalInput")
    yt = nc.dram_tensor("yt", [KT, P, C], dt.bfloat16, kind="ExternalOutput")

    with tile.TileContext(nc) as tc:
        with (
            tc.tile_pool(name="persist", bufs=1) as wpool,
            tc.tile_pool(name="gbuf", bufs=3) as gpool,
            tc.tile_pool(name="ybuf", bufs=4) as ypool,
            tc.tile_pool(name="silbuf", bufs=3) as spool,
            tc.tile_pool(name="psA", bufs=2, space="PSUM") as psA,
            tc.tile_pool(name="psB", bufs=3, space="PSUM") as psB,
        ):
            # SBUF layouts mirror the DRAM layouts so every DMA is contiguous
            # on both sides: per-chunk x tiles, m-major w1/w3.
            xss = [wpool.tile([P, KT, n], dt.bfloat16, name=f"xs{j}")
                   for j, n in enumerate(chunks)]
            w1s = wpool.tile([P, MT, KT * P], dt.bfloat16)
            w3s = wpool.tile([P, MT, KT * P], dt.bfloat16)
            w2s = wpool.tile([P, MT, DIM], dt.bfloat16)
            H = KT * P // 2
            # The head is ring-bandwidth-bound (~200 GB/s while ramping), so
            # what matters is enqueuing the m=0 critical bytes (w1 m0, x
            # chunk 0, w3 m0) with nothing else ahead of them. All inputs go
            # on the sync queue in consumption order; spreading them over two
            # queues just front-loads weight bytes ahead of x and delays the
            # first matmuls.
            nc.sync.dma_start(w1s[:, 0, :H], w1t[0][:, :H])
            nc.sync.dma_start(xss[0][:, :2, :], xts[0][:, :2, :])
            nc.sync.dma_start(w3s[:, 0, :H], w3t[0][:, :H])
            nc.sync.dma_start(xss[0][:, 2:4, :], xts[0][:, 2:4, :])
            nc.sync.dma_start(w1s[:, 0, H:], w1t[0][:, H:])
            nc.sync.dma_start(xss[0][:, 4:6, :], xts[0][:, 4:6, :])
            nc.sync.dma_start(w3s[:, 0, H:], w3t[0][:, H:])
            nc.sync.dma_start(xss[0][:, 6:, :], xts[0][:, 6:, :])
            for m in range(1, 4):
                nc.sync.dma_start(w1s[:, m, :], w1t[m])
                nc.sync.dma_start(w3s[:, m, :], w3t[m])
            # x chunk 1+ isn't consumed until its phase A (~60us in), so it
            # rides behind the remaining weights; putting it earlier delays
            # the m4-8 bulk and stalls the PE mid phase A.
            nc.sync.dma_start(w1s[:, 4:8, :], w1t[4:8].rearrange("m p q -> p m q"))
            nc.sync.dma_start(w3s[:, 4:8, :], w3t[4:8].rearrange("m p q -> p m q"))
            nc.sync.dma_start(w1s[:, 8:, :], w1t[8:].rearrange("m p q -> p m q"))
            nc.sync.dma_start(w3s[:, 8:, :], w3t[8:].rearrange("m p q -> p m q"))
            for j in range(1, len(chunks)):
                nc.sync.dma_start(xss[j][:], xts[j][:])
            nc.sync.dma_start(w2s[:, :6, :], w2t[:6].rearrange("m p q -> p m q"))
            nc.sync.dma_start(w2s[:, 6:, :], w2t[6:].rearrange("m p q -> p m q"))

            c0 = 0
            nchunks = len(chunks)
            for j, n in enumerate(chunks):
                xsj = xss[j]
                gs = gpool.tile([P, MT, n], dt.bfloat16, name="gs")
                for m in range(MT):
                    p1 = psA.tile([P, n], dt.float32, name="p1")
                    p3 = psA.tile([P, n], dt.float32, name="p3")
                    if j == 0 and m == 0:
                        # First tile pair: emit p1/p3 interleaved in the exact
                        # order their DMA dependencies land during the head
                        # ramp, so the PE computes through the ramp instead of
                        # idling until all of w1m0/w3m0/x0 has arrived.
                        seq = []
                        for kk in range(0, KT, 2):
                            seq += [(p1, w1s, kk), (p1, w1s, kk + 1),
                                    (p3, w3s, kk), (p3, w3s, kk + 1)]
                    else:
                        seq = [(p1, w1s, k) for k in range(KT)] + \
                              [(p3, w3s, k) for k in range(KT)]
                    seen = {}
                    for dst, wsrc, k in seq:
                        first = id(dst) not in seen
                        seen[id(dst)] = seen.get(id(dst), 0) + 1
                        nc.tensor.matmul(
                            dst[:],
                            wsrc[:, m, k * P:(k + 1) * P],
                            xsj[:, k, :],
                            start=first,
                            stop=(seen[id(dst)] == KT),
                        )
                    sil = spool.tile([P, n], dt.bfloat16, name="sil")
                    nc.scalar.activation(sil[:], p1[:], AF.Silu)
                    nc.vector.tensor_mul(gs[:, m, :], sil[:], p3[:])
                for i in range(KT):
                    # The very last output tile is split column-wise so the
                    # post-final-matmul chain (CAST + DMA trigger + ring
                    # transfer) only covers a small slice; the big slice
                    # drains while the small slice's matmuls run.
                    last = j == nchunks - 1 and i == KT - 1
                    TAIL = 232  # ~= LDWEIGHTS latency in matmul columns
                    splits = [(0, n - TAIL), (n - TAIL, n)] if (last and n - TAIL >= TAIL) else [(0, n)]
                    for si, (a, b) in enumerate(splits):
                        py = psB.tile([P, b - a], dt.float32, name="py")
                        for m in range(MT):
                            nc.tensor.matmul(
                                py[:],
                                w2s[:, m, i * P:(i + 1) * P],
                                gs[:, m, a:b],
                                start=(m == 0),
                                stop=(m == MT - 1),
                            )
                        if last and si == len(splits) - 1:
                            # Drain the very last tile via two parallel
                            # engine+queue chains to halve the post-matmul
                            # latency (cast + trigger + ring).
                            h = (b - a) // 2
                            ysa = ypool.tile([P, h], dt.bfloat16, name="ysa")
                            ysb = ypool.tile([P, b - a - h], dt.bfloat16, name="ysb")
                            nc.vector.tensor_copy(ysa[:], py[:, :h])
                            nc.scalar.activation(ysb[:], py[:, h:], AF.Copy)
                            nc.sync.dma_start(yt[i, :, c0 + a:c0 + a + h], ysa[:])
                            nc.scalar.dma_start(yt[i, :, c0 + a + h:c0 + b], ysb[:])
                        else:
                            ys = ypool.tile([P, b - a], dt.bfloat16, name="ys")
                            nc.vector.tensor_copy(ys[:], py[:])
                            q = nc.sync if (i + si) % 2 == 1 else nc.scalar
                            q.dma_start(yt[i, :, c0 + a:c0 + b], ys[:])
                c0 += n

    nc.compile()
    return nc


def _get_nc(C):
    if C not in _NC_CACHE:
        _NC_CACHE[C] = _build_nc(C)
    return _NC_CACHE[C]


def _ensure_ntff_hook_importable():
    # bass_utils imports antenv.axon_hooks when tracing is requested (e.g. via
    # a BASS_TRACE env var); in containers whose antenv stub lacks that
    # submodule the import would crash. Register a null hook so tracing just
    # degrades to "no trace" instead.
    import sys
    import types

    try:
        import antenv.axon_hooks  # noqa: F401
    except ImportError:
        mod = types.ModuleType("antenv.axon_hooks")
        mod.get_axon_ntff_profile_hook = lambda: None
        mod.set_axon_ntff_profile_hook = lambda hook: None
        sys.modules["antenv.axon_hooks"] = mod


def kernel(x, expert_indices, w1, w2, w3):
    global LAST_RESULTS
    import os
    import sys

    # The bass kernel executes on the NeuronCores via the axon PJRT backend;
    # a JAX_PLATFORMS=cpu pin (commonly used for running jax reference code)
    # would hide those devices. Clear it if jax hasn't initialized yet.
    if os.environ.get("JAX_PLATFORMS") == "cpu" and "jax" not in sys.modules:
        del os.environ["JAX_PLATFORMS"]

    from concourse import bass_utils

    _ensure_ntff_hook_importable()
    x = np.asarray(x, dtype=np.float32)
    idx = np.asarray(expert_indices)
    w1 = np.asarray(w1, dtype=np.float32)
    w2 = np.asarray(w2, dtype=np.float32)
    w3 = np.asarray(w3, dtype=np.float32)

    bf16 = ml_dtypes.bfloat16

    # --- host routing: stable-sort the (token, k) slots by expert id,
    # dropping slots whose (token, expert) pair duplicates slot k=0 ---
    flat = idx.reshape(-1).astype(np.int64)  # slot s = t*TOPK + k -> expert
    keep = np.ones(T * TOPK, dtype=bool)
    dup = idx[:, 1] == idx[:, 0]
    keep[1::2] = ~dup
    kept_slots = np.nonzero(keep)[0]
    kept_flat = flat[keep]
    order = np.argsort(kept_flat, kind="stable")  # kept slots grouped by expert
    sorted_slots = kept_slots[order]
    counts = np.bincount(kept_flat, minlength=E)
    starts = np.zeros(E + 1, dtype=np.int64)
    np.cumsum(counts, out=starts[1:])
    cmax = int(counts.max())
    C = max(256, -(-cmax // 8) * 8)  # pad capacity to a multiple of 8

    nc = _get_nc(C)

    chunks = _chunks_for(C)
    bounds = np.cumsum([0] + chunks)
    xb = x.astype(bf16)
    in_maps = []
    for e in range(E):
        slots = sorted_slots[starts[e]:starts[e + 1]]
        tokens = slots // TOPK
        xg = np.zeros((C, DIM), dtype=bf16)
        xg[: len(tokens)] = xb[tokens]
        # [C, DIM] -> [P, KT, C] (partition-major), then per-chunk blocks
        xpkc = xg.T.reshape(KT, P, C).transpose(1, 0, 2)
        im = {
            f"xt{j}": np.ascontiguousarray(xpkc[:, :, bounds[j]:bounds[j + 1]])
            for j in range(len(chunks))
        }
        # w1t[m, p, k*128+j] = w1[e][m*128+j, k*128+p]
        im["w1t"] = np.ascontiguousarray(
            w1[e].astype(bf16).reshape(MT, P, KT, P).transpose(0, 3, 2, 1)
        ).reshape(MT, P, KT * P)
        im["w3t"] = np.ascontiguousarray(
            w3[e].astype(bf16).reshape(MT, P, KT, P).transpose(0, 3, 2, 1)
        ).reshape(MT, P, KT * P)
        im["w2t"] = np.ascontiguousarray(w2[e].T.astype(bf16)).reshape(MT, P, DIM)
        in_maps.append(im)

    res = bass_utils.run_bass_kernel_spmd(
        nc, in_maps, core_ids=list(range(NCORES)), trace=TRACE
    )
    LAST_RESULTS = res

    out = np.empty((T * TOPK, DIM), dtype=np.float32)
    for e in range(E):
        slots = sorted_slots[starts[e]:starts[e + 1]]
        yt = res.results[e]["yt"]  # [KT, P, C] bf16
        y = yt.reshape(DIM, C).astype(np.float32)  # y.T
        out[slots] = y[:, : len(slots)].T
    out = out.reshape(T, TOPK, DIM)
    out[dup, 1] = out[dup, 0]  # slots dropped by dedupe share the k=0 result
    return out
